# revision 1
# baseline (speedup 1.0000x reference)
"""Trainium2 Bass kernel for the CGC multi-task MoE routing problem.

Full-input contract: kernel(**inputs) takes the unsharded numpy inputs and
returns the full [T+1, B, E] float32 output.

Strategy: pure data-parallel over batch across 8 NeuronCores (weights
replicated, no collectives). Per core (B_loc = 1024):
  - host pre-transposes activations to feature-major xT [D, B_loc] bf16
  - 16 experts (12 task-specific + 4 shared), each a 2-layer ReLU MLP,
    computed feature-major on TensorE in bf16:
        hT[H,B] = relu(W1.T @ xT + b1);  sT[E,B] = relu(W2.T @ hT + b2)
  - gate logits computed in [B, n_exp] orientation (lhsT = xT chunks),
    softmax along the free dim (exp on ScalarE with accum_out row-sum)
  - each expert output tile is PE-transposed to [B, E] and accumulated
    into per-task + shared-pool f32 accumulators with one fused
    scalar_tensor_tensor (acc = sT * gate + acc) per contribution
"""

import numpy as np
import ml_dtypes

import concourse.bass as bass
import concourse.mybir as mybir
from concourse.tile import TileContext
from concourse.bass_utils import run_bass_kernel_spmd

BF16 = ml_dtypes.bfloat16

# Problem shapes (hardcoded per spec)
T, B, D, H, E = 3, 8192, 1024, 512, 256
S, NS = 4, 4
NCORES = 8
BL = B // NCORES          # per-core batch rows (1024)
NBT = BL // 128           # b-tiles of 128 per core (8)
KD = D // 128             # contraction chunks for layer 1 (8)
KH = H // 128             # contraction chunks for layer 2 (4)
NEXP = T * S + NS         # 16 experts total
BN = 512                  # matmul moving free-dim chunk (1 PSUM bank of f32)
NBN = BL // BN            # 2

TRACE = False             # test harness sets kernel.TRACE = True for profiling
LAST_EXEC_NS = None

_CACHE = {}

# this walrus build rejects instructions carrying more than one semaphore wait
# condition ("Too many sync wait commands" in CoreV3 setupSyncWait; observed on
# Drain with 2+ and TensorTensor with 2), but Tile's sem-assigner and tail
# drain emit up to ~11 on one instruction
DRAIN_KEEP = 1
OTHER_KEEP = 1


def _split_excess_waits(nc):
    """Move overflow sem-waits onto same-engine NOPs inserted just before the
    overloaded instruction. Waiting earlier on the same engine preserves the
    ordering guarantee the wait provides."""
    n_split = 0
    for f in nc.m.functions:
        for bb in f.blocks:
            insts = bb.instructions
            need = False
            for i in insts:
                si = i.sync_info
                if si and si.on_wait and len(si.on_wait) > (
                    DRAIN_KEEP if isinstance(i, mybir.InstDrain) else OTHER_KEEP
                ):
                    need = True
                    break
            if not need:
                continue
            new_insts = []
            for inst in insts:
                si = inst.sync_info
                waits = list(si.on_wait) if si and si.on_wait else []
                keep = DRAIN_KEEP if isinstance(inst, mybir.InstDrain) else OTHER_KEEP
                if len(waits) > keep:
                    overflow = waits[: len(waits) - keep]
                    si.on_wait = waits[len(waits) - keep :]
                    for k, w in enumerate(overflow):
                        nop = mybir.InstNoOp(
                            name=f"{inst.name}-wsplit{k}", ins=[], outs=[]
                        )
                        nop.engine = inst.engine
                        nop.sync_info = mybir.SyncInfo(on_wait=[w], on_update=[])
                        new_insts.append(nop)
                        n_split += 1
                new_insts.append(inst)
            bb.instructions = new_insts
    return n_split


def _check_read_before_write(nc):
    """Emission-order lint: an on-chip tile read before any write means Tile
    will schedule the consumer against uninitialized memory (the bug class
    behind two earlier gate_sb/bg_sb ordering regressions)."""
    import sys

    written = set()
    flagged = set()
    for f in nc.m.functions:
        for bb in f.blocks:
            for inst in bb.instructions:
                for arg in inst.ins:
                    t = getattr(getattr(arg, "bass_ap", None), "tensor", None)
                    name = getattr(t, "name", None)
                    if name and name not in written and name not in flagged:
                        space = getattr(t, "space", None)
                        if str(space) in ("MemorySpace.SBUF", "MemorySpace.PSUM"):
                            flagged.add(name)
                            print(
                                f"WARNING: {inst.name} reads {name} before any "
                                f"write (emission order)",
                                file=sys.stderr,
                            )
                for arg in inst.outs:
                    t = getattr(getattr(arg, "bass_ap", None), "tensor", None)
                    name = getattr(t, "name", None)
                    if name:
                        written.add(name)


def _build_program(split_waits=True):
    f32 = mybir.dt.float32
    bf16 = mybir.dt.bfloat16
    relu = mybir.ActivationFunctionType.Relu
    expf = mybir.ActivationFunctionType.Exp
    mult = mybir.AluOpType.mult
    add = mybir.AluOpType.add

    nc = bass.Bass()
    xT = nc.dram_tensor("xT", [4, D, BL], bf16, kind="ExternalInput")
    w1 = nc.dram_tensor("w1", [NEXP, D, H], bf16, kind="ExternalInput")
    w2 = nc.dram_tensor("w2", [NEXP, H, E], bf16, kind="ExternalInput")
    b1 = nc.dram_tensor("b1", [NEXP, 128, KH], f32, kind="ExternalInput")
    b2 = nc.dram_tensor("b2", [NEXP, 128, E // 128], f32, kind="ExternalInput")
    wg = nc.dram_tensor("wg", [128, 4 * KD * 16], bf16, kind="ExternalInput")
    bg = nc.dram_tensor("bg", [128, 4 * 16], f32, kind="ExternalInput")
    ident = nc.dram_tensor("ident", [128, 128], bf16, kind="ExternalInput")
    out = nc.dram_tensor("out", [4, BL, E], f32, kind="ExternalOutput")

    with TileContext(nc) as tc:
        with (
            tc.tile_pool(name="const", bufs=1) as constp,
            tc.tile_pool(name="xp", bufs=1) as xp,
            tc.tile_pool(name="accp", bufs=1) as accp,
            tc.tile_pool(name="w1p", bufs=3) as w1p,
            tc.tile_pool(name="w2p", bufs=3) as w2p,
            tc.tile_pool(name="bp", bufs=4) as bp,
            tc.tile_pool(name="hp", bufs=3) as hp,
            tc.tile_pool(name="sp", bufs=3) as sp,
            tc.tile_pool(name="gp", bufs=4) as gp,
            tc.tile_pool(name="shp", bufs=8) as shp,
            tc.tile_pool(name="psh", bufs=3, space="PSUM") as psh_pool,
            tc.tile_pool(name="pss", bufs=2, space="PSUM") as pss_pool,
            tc.tile_pool(name="pst", bufs=2, space="PSUM") as pst_pool,
            tc.tile_pool(name="psg", bufs=1, space="PSUM") as psg_pool,
        ):
            wg_sb = constp.tile([128, 4 * KD * 16], bf16)
            bg_sb = constp.tile([128, 4 * 16], f32)
            id_sb = constp.tile([128, 128], bf16)

            # x tiles: chunked DMAs so consumers start as chunks land.
            # shared input (src 3) first: the shared gate is computed first.
            xt_sb = [
                xp.tile([128, KD * BL], bf16, name=f"xt{src}") for src in range(4)
            ]

            def load_xt(src):
                for c in range(KD):
                    nc.sync.dma_start(
                        out=xt_sb[src][:, c * BL : (c + 1) * BL],
                        in_=xT[src][c * 128 : (c + 1) * 128, :],
                    )

            acc = [accp.tile([128, NBT * E], f32, name=f"acc{t}") for t in range(4)]

            def next_pst():
                return pst_pool.tile([128, E], bf16, tag="pst", name="ps_t")
            gate_sb = [
                constp.tile([128, NBT * 16], f32, name=f"gate{s}") for s in range(4)
            ]
            written = set()  # (acc_idx, bt) already initialized

            def emit_gates(src):
                wexp = 8 if src < 3 else 16
                # one PSUM bank holds all NBT b-tiles of this gate set;
                # c-inner order so each xt chunk is consumed as it arrives
                # all NBT b-tiles form ONE psum accumulation group: start=True
                # lazily zeroes the whole 2KB zero-region, so each slice's
                # first write initializes it and later writes accumulate.
                # Interleaved per-slice groups would wipe siblings' partials.
                psg = psg_pool.tile([128, NBT * 16], f32)
                for c in range(KD):
                    for bt in range(NBT):
                        nc.tensor.matmul(
                            psg[:, bt * 16 : bt * 16 + 16],
                            lhsT=xt_sb[src][:, c * BL + bt * 128 : c * BL + bt * 128 + 128],
                            rhs=wg_sb[:, (src * KD + c) * 16 : (src * KD + c) * 16 + 16],
                            start=(c == 0 and bt == 0),
                            stop=(c == KD - 1 and bt == NBT - 1),
                        )
                # single fast eviction so the PSUM bank frees immediately;
                # softmax then runs off SBUF without stalling the next gate set
                logits = gp.tile([128, NBT * 16], f32, tag="logits")
                nc.scalar.copy(logits, psg)
                for bt in range(NBT):
                    logit = gp.tile([128, 16], f32, tag="logit")
                    nc.vector.tensor_add(
                        logit[:, :wexp],
                        logits[:, bt * 16 : bt * 16 + wexp],
                        bg_sb[:, src * 16 : src * 16 + wexp],
                    )
                    g_ap = gate_sb[src][:, bt * 16 : bt * 16 + wexp]
                    ssum = gp.tile([128, 1], f32, tag="ssum")
                    nc.scalar.activation(g_ap, logit[:, :wexp], expf, accum_out=ssum)
                    rsum = gp.tile([128, 1], f32, tag="rsum")
                    nc.vector.reciprocal(rsum, ssum)
                    nc.vector.tensor_scalar_mul(g_ap, g_ap, rsum)

            # SBUF staging for shared experts computed before the task gates
            # exist: their transposed [B,E] tiles wait here until the gates
            # are ready and the deferred combine runs
            stage = {}

            def load_w1(e):
                w1_sb = w1p.tile([128, KD * H], bf16)
                nc.sync.dma_start(
                    out=w1_sb.rearrange("p (c h) -> p c h", c=KD),
                    in_=w1[e].rearrange("(c p) h -> p c h", p=128),
                )
                return w1_sb

            DEFAULT_BN = [(0, BN), (BN, BN)]

            def emit_expert(e, src, finalize, defer=False, extra_per_bt=None,
                            w1_pre=None, h_pre=None, bn_list=DEFAULT_BN):
                w1_sb = w1_pre if w1_pre is not None else load_w1(e)
                w2_sb = w2p.tile([128, KH * E], bf16)
                nc.sync.dma_start(
                    out=w2_sb.rearrange("p (c f) -> p c f", c=KH),
                    in_=w2[e].rearrange("(c p) f -> p c f", p=128),
                )
                b1_sb = bp.tile([128, KH], f32, tag="b1")
                nc.sync.dma_start(out=b1_sb, in_=b1[e])
                b2_sb = bp.tile([128, E // 128], f32, tag="b2")
                nc.sync.dma_start(out=b2_sb, in_=b2[e])

                for off, W in bn_list:
                    if h_pre is not None and off == 0:
                        h_sb = h_pre
                    else:
                        h_sb = hp.tile([128, KH * W], bf16, name="h_sb", tag="h_sb")
                        for hc in range(KH):
                            ps_h = psh_pool.tile([128, W], f32, name="ps_h", tag="ps_h")
                            for c in range(KD):
                                nc.tensor.matmul(
                                    ps_h,
                                    lhsT=w1_sb[:, c * H + hc * 128 : c * H + hc * 128 + 128],
                                    rhs=xt_sb[src][:, c * BL + off : c * BL + off + W],
                                    start=(c == 0),
                                    stop=(c == KD - 1),
                                )
                            nc.scalar.activation(
                                h_sb[:, hc * W : (hc + 1) * W],
                                ps_h,
                                relu,
                                bias=b1_sb[:, hc : hc + 1],
                            )
                    s_sb = sp.tile([128, 2 * W], bf16, name="s_sb", tag="s_sb")
                    for ec in range(2):
                        ps_s = pss_pool.tile([128, W], f32, name="ps_s", tag="ps_s")
                        for hc in range(KH):
                            nc.tensor.matmul(
                                ps_s,
                                lhsT=w2_sb[:, hc * E + ec * 128 : hc * E + ec * 128 + 128],
                                rhs=h_sb[:, hc * W : (hc + 1) * W],
                                start=(hc == 0),
                                stop=(hc == KH - 1),
                            )
                        nc.scalar.activation(
                            s_sb[:, ec * W : (ec + 1) * W],
                            ps_s,
                            relu,
                            bias=b2_sb[:, ec : ec + 1],
                        )
                    for j in range(W // 128):
                        bt = off // 128 + j
                        ps_t = next_pst()
                        for ec in range(2):
                            nc.tensor.transpose(
                                ps_t[:, ec * 128 : (ec + 1) * 128],
                                s_sb[:, ec * W + j * 128 : ec * W + j * 128 + 128],
                                id_sb,
                            )

                        if defer:
                            st = shp.tile([128, E], bf16, tag=f"st{e}")
                            nc.scalar.copy(st, ps_t)
                            stage[(e, bt)] = st
                        else:
                            emit_contribs(e, bt, ps_t)
                        if extra_per_bt is not None:
                            extra_per_bt(bt)

                        # flush finished accumulator chunks to DRAM as soon as
                        # their last contribution lands (batched per chunk to
                        # amortize DMA descriptor latency)
                        if j == W // 128 - 1:
                            nb = W // 128
                            b0 = off // 128
                            for t in finalize:
                                nc.sync.dma_start(
                                    out=out[t][off : off + W, :].rearrange(
                                        "(b p) f -> p b f", p=128
                                    ),
                                    in_=acc[t][
                                        :, b0 * E : (b0 + nb) * E
                                    ].rearrange("p (b f) -> p b f", b=nb),
                                )

            def contribs_of(e):
                if e < T * S:
                    t, s = divmod(e, S)
                    return [(t, s), (3, t * S + s)]
                jsh = e - T * S
                return [(t, S + jsh) for t in range(T)] + [(3, T * S + jsh)]

            def emit_contribs(e, bt, src_tile):
                for gset, col in contribs_of(e):
                    g = gate_sb[gset][:, bt * 16 + col : bt * 16 + col + 1]
                    a = acc[gset][:, bt * E : (bt + 1) * E]
                    if (gset, bt) not in written:
                        written.add((gset, bt))
                        nc.vector.tensor_scalar_mul(a, src_tile, g)
                    else:
                        nc.vector.scalar_tensor_tensor(
                            a, src_tile, g, a, op0=mult, op1=add
                        )

            # Emission = engine program order. Shared experts first: they need
            # no gates at compute time (combine deferred via SBUF staging), so
            # PE ramps while only xt3 + their weights are in flight. Task
            # gates follow, then spec experts with the deferred shared-pool
            # contributions interleaved per b-tile. Tail = e11 (2 contribs).
            # acc0 ends with e3's hook (deferred e15), acc1 with e7,
            # acc2 / acc3 with e11
            finalize_at = {3: [0], 7: [1], 11: [2, 3]}

            # Prologue: xt3 and w1[12] chunk DMAs interleaved; e12's first-half
            # layer-1 runs c-outer across 3 PSUM banks so PE consumes each
            # (xt3, w1) chunk pair as it lands instead of idling on the load.
            w1_12 = w1p.tile([128, KD * H], bf16, name="w1_12")
            for c in range(KD):
                nc.sync.dma_start(
                    out=xt_sb[3][:, c * BL : (c + 1) * BL],
                    in_=xT[3][c * 128 : (c + 1) * 128, :],
                )
                nc.sync.dma_start(
                    out=w1_12[:, c * H : (c + 1) * H],
                    in_=w1[12][c * 128 : (c + 1) * 128, :],
                )
                if c == 1:
                    # mid-stream so it lands well before the gate(3) matmuls
                    # start, without delaying the first chunk pair
                    nc.sync.dma_start(out=wg_sb, in_=wg[:, :])
            b1_12 = bp.tile([128, KH], f32, tag="b1", name="b1_12")
            nc.sync.dma_start(out=b1_12, in_=b1[12])
            nc.sync.dma_start(out=bg_sb, in_=bg[:, :])
            nc.sync.dma_start(out=id_sb, in_=ident[:, :])

            h12 = hp.tile([128, KH * BN], bf16, name="h12")
            ph = [
                psh_pool.tile([128, BN], f32, name=f"ph{hc}", tag="ps_h")
                for hc in range(3)
            ]
            for c in range(KD):
                for hc in range(3):
                    nc.tensor.matmul(
                        ph[hc],
                        lhsT=w1_12[:, c * H + hc * 128 : c * H + hc * 128 + 128],
                        rhs=xt_sb[3][:, c * BL : c * BL + BN],
                        start=(c == 0),
                        stop=(c == KD - 1),
                    )
            for hc in range(3):
                nc.scalar.activation(
                    h12[:, hc * BN : (hc + 1) * BN], ph[hc], relu,
                    bias=b1_12[:, hc : hc + 1],
                )
            ph3 = psh_pool.tile([128, BN], f32, name="ph3", tag="ps_h")
            for c in range(KD):
                nc.tensor.matmul(
                    ph3,
                    lhsT=w1_12[:, c * H + 3 * 128 : c * H + 3 * 128 + 128],
                    rhs=xt_sb[3][:, c * BL : c * BL + BN],
                    start=(c == 0),
                    stop=(c == KD - 1),
                )
            nc.scalar.activation(
                h12[:, 3 * BN : 4 * BN], ph3, relu, bias=b1_12[:, 3:4]
            )
            emit_gates(3)
            emit_expert(12, 3, [], defer=True, w1_pre=w1_12, h_pre=h12)
            load_xt(0)
            emit_gates(0)
            emit_expert(13, 3, [], defer=True)
            emit_expert(14, 3, [], defer=True)
            load_xt(1)
            emit_gates(1)
            emit_expert(15, 3, [], defer=True)
            load_xt(2)
            emit_gates(2)

            def make_hook(shared_e):
                def hook(bt):
                    emit_contribs(shared_e, bt, stage[(shared_e, bt)])
                return hook

            for e in [0, 1, 2, 3]:
                emit_expert(e, 0, finalize_at.get(e, []),
                            extra_per_bt=make_hook(12 + e))
            for e in [4, 5, 6, 7]:
                emit_expert(e, 1, finalize_at.get(e, []))
            for e in [8, 9, 10]:
                emit_expert(e, 2, finalize_at.get(e, []))
            # last expert runs progressively finer column chunks so the final
            # combine + accumulator flush pipeline covers only 128 rows
            emit_expert(11, 2, finalize_at[11],
                        bn_list=[(0, 512), (512, 256), (768, 128), (896, 128)])

    _check_read_before_write(nc)
    if split_waits:
        _split_excess_waits(nc)
    return nc


def _prep_shared(W_spec1, b_spec1, W_spec2, b_spec2, W_sh1, b_sh1, W_sh2, b_sh2,
                 W_gate, b_gate, W_gate_sh, b_gate_sh):
    """Host-side prep of the replicated (per-core-identical) tensors."""
    w1 = np.ascontiguousarray(
        np.concatenate([W_spec1, W_sh1], axis=0).astype(BF16)
    )
    w2 = np.ascontiguousarray(
        np.concatenate([W_spec2, W_sh2], axis=0).astype(BF16)
    )
    b1 = np.ascontiguousarray(
        np.concatenate([b_spec1, b_sh1], axis=0)
        .astype(np.float32)
        .reshape(NEXP, KH, 128)
        .transpose(0, 2, 1)
    )
    b2 = np.ascontiguousarray(
        np.concatenate([b_spec2, b_sh2], axis=0)
        .astype(np.float32)
        .reshape(NEXP, E // 128, 128)
        .transpose(0, 2, 1)
    )
    # gate weights: [4 sets, D, 16] (task gates padded 8 -> 16 with zeros)
    wg_full = np.zeros((4, D, 16), np.float32)
    wg_full[:T, :, : S + NS] = W_gate
    wg_full[3] = W_gate_sh
    # device layout [128, (set, chunk, e)]
    wg = np.ascontiguousarray(
        wg_full.reshape(4, KD, 128, 16).transpose(2, 0, 1, 3).reshape(128, 4 * KD * 16)
    ).astype(BF16)
    bg_full = np.zeros((4, 16), np.float32)
    bg_full[:T, : S + NS] = b_gate
    bg_full[3] = b_gate_sh
    bg = np.ascontiguousarray(
        np.broadcast_to(bg_full[None], (128, 4, 16)).reshape(128, 64)
    )
    ident = np.eye(128, dtype=BF16)
    return dict(w1=w1, w2=w2, b1=b1, b2=b2, wg=wg, bg=bg, ident=ident)


def kernel(x_tasks, x_shared, W_spec1, b_spec1, W_spec2, b_spec2,
           W_sh1, b_sh1, W_sh2, b_sh2, W_gate, b_gate, W_gate_sh, b_gate_sh):
    global LAST_EXEC_NS
    if "nc" not in _CACHE:
        _CACHE["nc"] = _build_program()
    nc = _CACHE["nc"]

    shared = _prep_shared(W_spec1, b_spec1, W_spec2, b_spec2, W_sh1, b_sh1,
                          W_sh2, b_sh2, W_gate, b_gate, W_gate_sh, b_gate_sh)

    x_tasks = np.asarray(x_tasks, np.float32)
    x_shared = np.asarray(x_shared, np.float32)

    in_maps = []
    for i in range(NCORES):
        sl = slice(i * BL, (i + 1) * BL)
        xt = np.empty((4, D, BL), BF16)
        for t in range(T):
            xt[t] = x_tasks[t, sl, :].T.astype(BF16)
        xt[3] = x_shared[sl, :].T.astype(BF16)
        m = {"xT": xt}
        m.update(shared)
        in_maps.append(m)

    res = run_bass_kernel_spmd(nc, in_maps, core_ids=list(range(NCORES)), trace=TRACE)
    LAST_EXEC_NS = res.exec_time_ns

    full = np.empty((T + 1, B, E), np.float32)
    for i in range(NCORES):
        full[:, i * BL : (i + 1) * BL, :] = res.results[i]["out"]
    return full



# revision 24
# speedup vs baseline: 1.0688x; 1.0688x over previous
"""Trainium2 Bass kernel for the CGC multi-task MoE routing problem.

Full-input contract: kernel(**inputs) takes the unsharded numpy inputs and
returns the full [T+1, B, E] float32 output.

Strategy: pure data-parallel over batch across 8 NeuronCores (weights
replicated, no collectives). Per core (B_loc = 1024):
  - host pre-transposes activations to feature-major xT [D, B_loc] bf16
  - 16 experts (12 task-specific + 4 shared), each a 2-layer ReLU MLP:
        layer 1 feature-major on TensorE:  hT[H,B] = relu(W1.T @ xT + b1)
        layer 2 batch-major:               s[B,E]  = relu(hT.T @ W2)
    (hT is exactly the lhsT layer 2 needs, so expert outputs land in
    [B, E] orientation with no PE transposes at all)
  - gate logits computed in [B, n_exp] orientation (lhsT = xT chunks),
    softmax along the free dim (exp on ScalarE with accum_out row-sum)
  - each expert [B,E] tile is accumulated into per-task + shared-pool f32
    accumulators with one fused scalar_tensor_tensor (acc = s * gate + acc)
    per contribution
  - emission order keeps PE continuously fed: shared experts first (their
    gate-combine is deferred via SBUF staging), input/weight DMAs ordered
    just-in-time on the SP queue, output flushes on the Activation queue
"""

import numpy as np
import ml_dtypes

import concourse.bass as bass
import concourse.mybir as mybir
from concourse.tile import TileContext
from concourse.bass_utils import run_bass_kernel_spmd

BF16 = ml_dtypes.bfloat16

# Problem shapes (hardcoded per spec)
T, B, D, H, E = 3, 8192, 1024, 512, 256
S, NS = 4, 4
NCORES = 8
BL = B // NCORES          # per-core batch rows (1024)
NBT = BL // 128           # b-tiles of 128 per core (8)
KD = D // 128             # contraction chunks for layer 1 (8)
KH = H // 128             # contraction chunks for layer 2 (4)
NEXP = T * S + NS         # 16 experts total
BN = 512                  # layer-1 moving free-dim chunk (1 PSUM bank of f32)
NBN = BL // BN            # 2

TRACE = False             # test harness sets kernel.TRACE = True for profiling
LAST_EXEC_NS = None

_CACHE = {}

# this walrus build rejects instructions carrying more than one semaphore wait
# condition ("Too many sync wait commands" in CoreV3 setupSyncWait; observed on
# Drain with 2+ and TensorTensor with 2), but Tile's sem-assigner and tail
# drain emit up to ~11 on one instruction
DRAIN_KEEP = 1
OTHER_KEEP = 1


def _split_excess_waits(nc):
    """Move overflow sem-waits onto same-engine NOPs inserted just before the
    overloaded instruction. Waiting earlier on the same engine preserves the
    ordering guarantee the wait provides."""
    n_split = 0
    for f in nc.m.functions:
        for bb in f.blocks:
            insts = bb.instructions
            need = False
            for i in insts:
                si = i.sync_info
                if si and si.on_wait and len(si.on_wait) > (
                    DRAIN_KEEP if isinstance(i, mybir.InstDrain) else OTHER_KEEP
                ):
                    need = True
                    break
            if not need:
                continue
            new_insts = []
            for inst in insts:
                si = inst.sync_info
                waits = list(si.on_wait) if si and si.on_wait else []
                keep = DRAIN_KEEP if isinstance(inst, mybir.InstDrain) else OTHER_KEEP
                if len(waits) > keep:
                    overflow = waits[: len(waits) - keep]
                    si.on_wait = waits[len(waits) - keep :]
                    for k, w in enumerate(overflow):
                        nop = mybir.InstNoOp(
                            name=f"{inst.name}-wsplit{k}", ins=[], outs=[]
                        )
                        nop.engine = inst.engine
                        nop.sync_info = mybir.SyncInfo(on_wait=[w], on_update=[])
                        new_insts.append(nop)
                        n_split += 1
                new_insts.append(inst)
            bb.instructions = new_insts
    return n_split


def _check_read_before_write(nc):
    """Emission-order lint: an on-chip tile read before any write means Tile
    will schedule the consumer against uninitialized memory (the bug class
    behind two earlier gate_sb/bg_sb ordering regressions)."""
    import sys

    written = set()
    flagged = set()
    for f in nc.m.functions:
        for bb in f.blocks:
            for inst in bb.instructions:
                for arg in inst.ins:
                    t = getattr(getattr(arg, "bass_ap", None), "tensor", None)
                    name = getattr(t, "name", None)
                    if name and name not in written and name not in flagged:
                        space = getattr(t, "space", None)
                        if str(space) in ("MemorySpace.SBUF", "MemorySpace.PSUM"):
                            flagged.add(name)
                            print(
                                f"WARNING: {inst.name} reads {name} before any "
                                f"write (emission order)",
                                file=sys.stderr,
                            )
                for arg in inst.outs:
                    t = getattr(getattr(arg, "bass_ap", None), "tensor", None)
                    name = getattr(t, "name", None)
                    if name:
                        written.add(name)


def _build_program(with_b2=False, split_waits=True):
    f32 = mybir.dt.float32
    bf16 = mybir.dt.bfloat16
    relu = mybir.ActivationFunctionType.Relu
    expf = mybir.ActivationFunctionType.Exp
    mult = mybir.AluOpType.mult
    add = mybir.AluOpType.add

    nc = bass.Bass()
    xT = nc.dram_tensor("xT", [4, D, BL], bf16, kind="ExternalInput")
    w1 = nc.dram_tensor("w1", [NEXP, D, H], bf16, kind="ExternalInput")
    w2 = nc.dram_tensor("w2", [NEXP, H, E], bf16, kind="ExternalInput")
    b1 = nc.dram_tensor("b1", [128, NEXP * KH], f32, kind="ExternalInput")
    wg = nc.dram_tensor("wg", [128, 4 * KD * 16], bf16, kind="ExternalInput")
    bg = nc.dram_tensor("bg", [128, 4 * 16], f32, kind="ExternalInput")
    if with_b2:
        b2bc = nc.dram_tensor("b2bc", [128, NEXP * E], f32, kind="ExternalInput")
    out = nc.dram_tensor("out", [4, BL, E], f32, kind="ExternalOutput")

    with TileContext(nc) as tc:
        with (
            tc.tile_pool(name="const", bufs=1) as constp,
            tc.tile_pool(name="xp", bufs=1) as xp,
            tc.tile_pool(name="accp", bufs=1) as accp,
            tc.tile_pool(name="w1p", bufs=4) as w1p,
            tc.tile_pool(name="w2p", bufs=4) as w2p,
            tc.tile_pool(name="hp", bufs=3) as hp,
            tc.tile_pool(name="sp", bufs=4) as sp,
            tc.tile_pool(name="gp", bufs=4) as gp,
            tc.tile_pool(name="shp", bufs=8) as shp,
            tc.tile_pool(name="psh", bufs=3, space="PSUM") as psh_pool,
            tc.tile_pool(name="pss", bufs=5, space="PSUM") as pss_pool,
        ):
            wg_sb = constp.tile([128, 4 * KD * 16], bf16)
            bg_sb = constp.tile([128, 4 * 16], f32)
            b1_sb = constp.tile([128, NEXP * KH], f32)
            if with_b2:
                b2_sb = constp.tile([128, NEXP * E], f32)

            # x tiles: chunked DMAs so consumers start as chunks land
            xt_sb = [
                xp.tile([128, KD * BL], bf16, name=f"xt{src}") for src in range(4)
            ]

            def load_xt(src, half=None):
                # one DMA per 4-chunk half: HWDGE descriptor processing is a
                # shared ~625ns/DMA serial resource, so fewer+bigger wins
                for h in ([half] if half is not None else (0, 1)):
                    c0 = h * (KD // 2)
                    nc.sync.dma_start(
                        out=xt_sb[src][:, c0 * BL : (c0 + KD // 2) * BL].rearrange(
                            "p (c b) -> p c b", c=KD // 2
                        ),
                        in_=xT[src][c0 * 128 : (c0 + KD // 2) * 128, :].rearrange(
                            "(c p) b -> p c b", p=128
                        ),
                    )

            acc = [accp.tile([128, NBT * E], f32, name=f"acc{t}") for t in range(4)]

            gate_sb = [
                constp.tile([128, NBT * 16], f32, name=f"gate{s}") for s in range(4)
            ]
            written = set()  # (acc_idx, bt) already initialized

            def emit_gates(src):
                wexp = 8 if src < 3 else 16
                # one psg region holds all NBT b-tiles of this gate set as a
                # single accumulation group: start=True lazily zeroes the whole
                # zero-region, each slice's first write initializes it and
                # later writes accumulate. c-inner so each xt chunk is
                # consumed as it arrives.
                # gate psums rotate through the shared pss bank pool so two
                # gate sets hoisted together by the scheduler never contend
                # on a single dedicated bank
                psg = pss_pool.tile([128, NBT * wexp], f32, name="psg", tag="ps_s")
                for c in range(KD):
                    for bt in range(NBT):
                        nc.tensor.matmul(
                            psg[:, bt * wexp : bt * wexp + wexp],
                            lhsT=xt_sb[src][:, c * BL + bt * 128 : c * BL + bt * 128 + 128],
                            rhs=wg_sb[:, (src * KD + c) * 16 : (src * KD + c) * 16 + wexp],
                            start=(c == 0 and bt == 0),
                            stop=(c == KD - 1 and bt == NBT - 1),
                        )
                # single fast eviction so the PSUM region frees immediately;
                # softmax then runs off SBUF
                logits = gp.tile([128, NBT * wexp], f32, tag="logits")
                nc.scalar.copy(logits, psg)
                for bt in range(NBT):
                    logit = gp.tile([128, 16], f32, tag="logit")
                    nc.vector.tensor_add(
                        logit[:, :wexp],
                        logits[:, bt * wexp : bt * wexp + wexp],
                        bg_sb[:, src * 16 : src * 16 + wexp],
                    )
                    g_ap = gate_sb[src][:, bt * 16 : bt * 16 + wexp]
                    ssum = gp.tile([128, 1], f32, tag="ssum")
                    nc.scalar.activation(g_ap, logit[:, :wexp], expf, accum_out=ssum)
                    rsum = gp.tile([128, 1], f32, tag="rsum")
                    nc.vector.reciprocal(rsum, ssum)
                    nc.vector.tensor_scalar_mul(g_ap, g_ap, rsum)

            # SBUF staging for shared experts computed before the task gates
            # exist: their [B,E] tiles wait here until the gates are ready and
            # the deferred combine runs
            stage = {}

            def load_w1(e):
                w1_sb = w1p.tile([128, KD * H], bf16, name="w1_sb", tag="w1_sb")
                nc.sync.dma_start(
                    out=w1_sb.rearrange("p (c h) -> p c h", c=KD),
                    in_=w1[e].rearrange("(c p) h -> p c h", p=128),
                )
                return w1_sb

            def load_w2(e):
                w2_sb = w2p.tile([128, KH * E], bf16, name="w2_sb", tag="w2_sb")
                nc.sync.dma_start(
                    out=w2_sb.rearrange("p (c f) -> p c f", c=KH),
                    in_=w2[e].rearrange("(c p) f -> p c f", p=128),
                )
                return w2_sb

            def contribs_of(e):
                if e < T * S:
                    t, s = divmod(e, S)
                    return [(t, s), (3, t * S + s)]
                jsh = e - T * S
                return [(t, S + jsh) for t in range(T)] + [(3, T * S + jsh)]

            def emit_contrib(gset, col, bt, src_tile):
                # combines must live on DVE: it is the only engine whose ISA
                # has the fused scalar_tensor_tensor (acc = s*g + acc), and
                # Pool cannot even read PSUM on real TRN2
                g = gate_sb[gset][:, bt * 16 + col : bt * 16 + col + 1]
                a = acc[gset][:, bt * E : (bt + 1) * E]
                if (gset, bt) not in written:
                    written.add((gset, bt))
                    nc.vector.tensor_scalar_mul(a, src_tile, g)
                else:
                    nc.vector.scalar_tensor_tensor(
                        a, src_tile, g, a, op0=mult, op1=add
                    )

            def emit_contribs(e, bt, src_tile):
                for gset, col in contribs_of(e):
                    emit_contrib(gset, col, bt, src_tile)

            def flush(t, row0, nrows):
                nb = nrows // 128
                b0 = row0 // 128
                # output flushes ride the Activation-engine HWDGE queue so a
                # flush waiting on combines never head-of-line-blocks input
                # weight/x loads on the SP queue
                nc.sync.dma_start(
                    out=out[t][row0 : row0 + nrows, :].rearrange(
                        "(b p) f -> p b f", p=128
                    ),
                    in_=acc[t][:, b0 * E : (b0 + nb) * E].rearrange(
                        "p (b f) -> p b f", b=nb
                    ),
                )

            DEFAULT_BN = [(0, BN), (BN, BN)]
            # last expert: progressively finer granules so only ONE b-tile's
            # evict+combine+flush chain drains after the last matmul
            TAIL_BN = [(0, BN), (BN, BN // 2), (768, 128), (896, 128)]

            def emit_expert(e, src, finalize, defer=False, extra_per_bt=None,
                            w1_pre=None, h_pre=None, w2_pre=None,
                            fine_tail=False):
                w1_sb = w1_pre if w1_pre is not None else load_w1(e)
                w2_sb = w2_pre if w2_pre is not None else load_w2(e)

                for off, W in (TAIL_BN if fine_tail else DEFAULT_BN):
                    if h_pre is not None and off == 0:
                        h_sb = h_pre
                    else:
                        h_sb = hp.tile([128, KH * W], bf16, name="h_sb", tag="h_sb")
                        for hc in range(KH):
                            ps_h = psh_pool.tile([128, W], f32, name="ps_h", tag="ps_h")
                            for c in range(KD):
                                nc.tensor.matmul(
                                    ps_h,
                                    lhsT=w1_sb[:, c * H + hc * 128 : c * H + hc * 128 + 128],
                                    rhs=xt_sb[src][:, c * BL + off : c * BL + off + W],
                                    start=(c == 0),
                                    stop=(c == KD - 1),
                                )
                            nc.scalar.activation(
                                h_sb[:, hc * W : (hc + 1) * W],
                                ps_h,
                                relu,
                                bias=b1_sb[:, e * KH + hc : e * KH + hc + 1],
                            )
                    for j in range(W // 128):
                        bt = off // 128 + j
                        ps_s = pss_pool.tile([128, E], f32, name="ps_s", tag="ps_s")
                        for hc in range(KH):
                            nc.tensor.matmul(
                                ps_s,
                                lhsT=h_sb[:, hc * W + j * 128 : hc * W + j * 128 + 128],
                                rhs=w2_sb[:, hc * E : (hc + 1) * E],
                                start=(hc == 0),
                                stop=(hc == KH - 1),
                            )
                        if with_b2:
                            nc.vector.tensor_add(
                                ps_s, ps_s, b2_sb[:, e * E : (e + 1) * E]
                            )
                        if defer:
                            st = shp.tile([128, E], bf16, name=f"st{e}", tag=f"st{e}")
                            nc.scalar.activation(st, ps_s, relu)
                            stage[(e, bt)] = st
                            if extra_per_bt is not None:
                                extra_per_bt(bt)
                        else:
                            s_sb = sp.tile([128, E], bf16, name="s_sb", tag="s_sb")
                            nc.scalar.activation(s_sb, ps_s, relu)
                            # deferred shared-expert combines first: they only
                            # need staged tiles + gates, so they never sit on
                            # the critical tail chain
                            if extra_per_bt is not None:
                                extra_per_bt(bt)
                            emit_contribs(e, bt, s_sb)

                        # flush finished accumulator rows to DRAM as soon as
                        # their last contribution lands (per bn/granule chunk)
                        if j == W // 128 - 1:
                            for t in finalize:
                                flush(t, off, W)

            # Emission = per-engine program order. Shared experts first: they
            # need no gates at compute time (combine deferred via SBUF
            # staging), so PE ramps while only xt3 + their weights are in
            # flight. gates(3) follows expert 12 (nothing needs it earlier),
            # then the remaining shared experts, then task phases 0/1/2 with
            # the deferred shared-pool contributions interleaved per b-tile.
            finalize_at = {3: [0], 7: [1], 11: [2, 3]}

            # DMA prologue. One expert's layer-1 eats (w1 + x-half) at
            # ~600GB/s while DMA sustains ~330GB/s — so the first TWO shared
            # experts' bn0 layer-1s are interleaved c-outer: the x chunks
            # amortize over both and the combined demand (~300GB/s) fits the
            # link. Pieces sized 1-2-2-3 chunks: small first for an early PE
            # start, then big enough to amortize the shared ~625ns/DMA HWDGE
            # descriptor cost.
            w1_12 = w1p.tile([128, KD * H], bf16, name="w1_sb", tag="w1_sb")
            w1_13 = w1p.tile([128, KD * H], bf16, name="w1_sb", tag="w1_sb")

            def w1_piece(tile, e, c0, c1):
                nc.sync.dma_start(
                    out=tile[:, c0 * H : c1 * H].rearrange(
                        "p (c h) -> p c h", c=c1 - c0
                    ),
                    in_=w1[e][c0 * 128 : c1 * 128, :].rearrange(
                        "(c p) h -> p c h", p=128
                    ),
                )

            def xt3_piece(c0, c1, b0, b1_):
                nc.sync.dma_start(
                    out=xt_sb[3][:, c0 * BL : c1 * BL].rearrange(
                        "p (c b) -> p c b", c=c1 - c0
                    )[:, :, b0:b1_],
                    in_=xT[3][c0 * 128 : c1 * 128, b0:b1_].rearrange(
                        "(c p) b -> p c b", p=128
                    ),
                )

            for c0, c1 in ((0, 1), (1, 3), (3, 5), (5, 8)):
                w1_piece(w1_12, 12, c0, c1)
                xt3_piece(c0, c1, 0, BN)
                w1_piece(w1_13, 13, c0, c1)
                if c0 == 0:
                    nc.sync.dma_start(out=b1_sb, in_=b1[:, :])
            nc.sync.dma_start(out=wg_sb, in_=wg[:, :])
            w2_12 = load_w2(12)
            w2_13 = load_w2(13)
            xt3_piece(0, 8, BN, BL)
            nc.sync.dma_start(out=bg_sb, in_=bg[:, :])
            if with_b2:
                nc.sync.dma_start(out=b2_sb, in_=b2bc[:, :])

            # PE prologue: experts 12+13's first-half layer-1 interleaved
            # c-outer across 6 PSUM banks (3 psh + 3 pss) so PE consumes each
            # (w1, w1', x) chunk triple as it lands; the 4th h-chunks run as
            # second passes from the two spare pss banks (no eviction wait).
            h12 = hp.tile([128, KH * BN], bf16, name="h12")
            h13 = hp.tile([128, KH * BN], bf16, name="h13")
            ph12 = [
                psh_pool.tile([128, BN], f32, name=f"ph12_{hc}", tag="ps_h")
                for hc in range(3)
            ]
            ph13 = [
                pss_pool.tile([128, BN], f32, name=f"ph13_{hc}", tag="ps_s")
                for hc in range(3)
            ]
            for c in range(KD):
                for ph_l, w1_l in ((ph12, w1_12), (ph13, w1_13)):
                    for hc in range(3):
                        nc.tensor.matmul(
                            ph_l[hc],
                            lhsT=w1_l[:, c * H + hc * 128 : c * H + hc * 128 + 128],
                            rhs=xt_sb[3][:, c * BL : c * BL + BN],
                            start=(c == 0),
                            stop=(c == KD - 1),
                        )
            for h_l, ph_l, e in ((h12, ph12, 12), (h13, ph13, 13)):
                for hc in range(3):
                    nc.scalar.activation(
                        h_l[:, hc * BN : (hc + 1) * BN], ph_l[hc], relu,
                        bias=b1_sb[:, e * KH + hc : e * KH + hc + 1],
                    )
            for h_l, w1_l, e in ((h12, w1_12, 12), (h13, w1_13, 13)):
                ph3 = pss_pool.tile([128, BN], f32, name="ph3", tag="ps_s")
                for c in range(KD):
                    nc.tensor.matmul(
                        ph3,
                        lhsT=w1_l[:, c * H + 3 * 128 : c * H + 3 * 128 + 128],
                        rhs=xt_sb[3][:, c * BL : c * BL + BN],
                        start=(c == 0),
                        stop=(c == KD - 1),
                    )
                nc.scalar.activation(
                    h_l[:, 3 * BN : 4 * BN], ph3, relu,
                    bias=b1_sb[:, e * KH + 3 : e * KH + 4],
                )

            emit_expert(12, 3, [], defer=True, w1_pre=w1_12, h_pre=h12,
                        w2_pre=w2_12)
            emit_gates(3)
            emit_expert(13, 3, [], defer=True, w1_pre=w1_13, h_pre=h13,
                        w2_pre=w2_13)
            emit_expert(14, 3, [], defer=True)
            load_xt(0)
            emit_expert(15, 3, [], defer=True)
            emit_gates(0)

            # deferred shared-expert combines, spread across the task phases:
            # st_{12+k} -> acc_t runs during phase t's k-th expert (so
            # gates(1)/gates(2) aren't needed until their own phase), and
            # st_{12+k} -> acc3 runs during phase 0.
            def make_hook(shared_e, phase):
                def hook(bt):
                    emit_contrib(phase, S + (shared_e - T * S), bt,
                                 stage[(shared_e, bt)])
                    if phase == 0:
                        emit_contrib(3, T * S + (shared_e - T * S), bt,
                                     stage[(shared_e, bt)])
                return hook

            for k, e in enumerate([0, 1, 2, 3]):
                emit_expert(e, 0, finalize_at.get(e, []),
                            extra_per_bt=make_hook(12 + k, 0))
                if e == 1:
                    load_xt(1, half=0)
                if e == 2:
                    load_xt(1, half=1)
            emit_gates(1)
            for k, e in enumerate([4, 5, 6, 7]):
                emit_expert(e, 1, finalize_at.get(e, []),
                            extra_per_bt=make_hook(12 + k, 1))
                if e == 5:
                    load_xt(2, half=0)
                if e == 6:
                    load_xt(2, half=1)
            emit_gates(2)
            for k, e in enumerate([8, 9, 10, 11]):
                emit_expert(e, 2, finalize_at.get(e, []),
                            extra_per_bt=make_hook(12 + k, 2),
                            fine_tail=(e == 11))

    _check_read_before_write(nc)
    if split_waits:
        _split_excess_waits(nc)
    return nc


def _prep_shared(W_spec1, b_spec1, W_spec2, b_spec2, W_sh1, b_sh1, W_sh2, b_sh2,
                 W_gate, b_gate, W_gate_sh, b_gate_sh, with_b2):
    """Host-side prep of the replicated (per-core-identical) tensors."""
    w1 = np.ascontiguousarray(
        np.concatenate([W_spec1, W_sh1], axis=0).astype(BF16)
    )
    w2 = np.ascontiguousarray(
        np.concatenate([W_spec2, W_sh2], axis=0).astype(BF16)
    )
    # layer-1 biases, batched: [128, e*KH + hc] (partition = h within chunk)
    b1 = np.ascontiguousarray(
        np.concatenate([b_spec1, b_sh1], axis=0)
        .astype(np.float32)
        .reshape(NEXP, KH, 128)
        .transpose(2, 0, 1)
        .reshape(128, NEXP * KH)
    )
    # gate weights: [4 sets, D, 16] (task gates padded 8 -> 16 with zeros)
    wg_full = np.zeros((4, D, 16), np.float32)
    wg_full[:T, :, : S + NS] = W_gate
    wg_full[3] = W_gate_sh
    # device layout [128, (set, chunk, e)]
    wg = np.ascontiguousarray(
        wg_full.reshape(4, KD, 128, 16).transpose(2, 0, 1, 3).reshape(128, 4 * KD * 16)
    ).astype(BF16)
    bg_full = np.zeros((4, 16), np.float32)
    bg_full[:T, : S + NS] = b_gate
    bg_full[3] = b_gate_sh
    bg = np.ascontiguousarray(
        np.broadcast_to(bg_full[None], (128, 4, 16)).reshape(128, 64)
    )
    res = dict(w1=w1, w2=w2, b1=b1, wg=wg, bg=bg)
    if with_b2:
        b2_full = np.concatenate([b_spec2, b_sh2], axis=0).astype(np.float32)
        res["b2bc"] = np.ascontiguousarray(
            np.broadcast_to(b2_full.reshape(1, NEXP * E), (128, NEXP * E))
        )
    return res


def kernel(x_tasks, x_shared, W_spec1, b_spec1, W_spec2, b_spec2,
           W_sh1, b_sh1, W_sh2, b_sh2, W_gate, b_gate, W_gate_sh, b_gate_sh):
    global LAST_EXEC_NS
    with_b2 = bool(np.any(np.asarray(b_spec2)) or np.any(np.asarray(b_sh2)))
    key = ("nc", with_b2)
    if key not in _CACHE:
        _CACHE[key] = _build_program(with_b2=with_b2)
    nc = _CACHE[key]
    _CACHE["nc"] = nc  # latest program, for the test harness's TimelineSim

    shared = _prep_shared(W_spec1, b_spec1, W_spec2, b_spec2, W_sh1, b_sh1,
                          W_sh2, b_sh2, W_gate, b_gate, W_gate_sh, b_gate_sh,
                          with_b2)

    x_tasks = np.asarray(x_tasks, np.float32)
    x_shared = np.asarray(x_shared, np.float32)

    in_maps = []
    for i in range(NCORES):
        sl = slice(i * BL, (i + 1) * BL)
        xt = np.empty((4, D, BL), BF16)
        for t in range(T):
            xt[t] = x_tasks[t, sl, :].T.astype(BF16)
        xt[3] = x_shared[sl, :].T.astype(BF16)
        m = {"xT": xt}
        m.update(shared)
        in_maps.append(m)

    res = run_bass_kernel_spmd(nc, in_maps, core_ids=list(range(NCORES)), trace=TRACE)
    LAST_EXEC_NS = res.exec_time_ns

    full = np.empty((T + 1, B, E), np.float32)
    for i in range(NCORES):
        full[:, i * BL : (i + 1) * BL, :] = res.results[i]["out"]
    return full


# revision 32
# speedup vs baseline: 1.2908x; 1.2077x over previous
"""Trainium2 Bass kernel for the CGC multi-task MoE routing problem.

Full-input contract: kernel(**inputs) takes the unsharded numpy inputs and
returns the full [T+1, B, E] float32 output.

Strategy: pure data-parallel over batch across 8 NeuronCores (weights
replicated, no collectives). Per core (B_loc = 1024):
  - host pre-transposes activations to feature-major xT [D, B_loc]
  - 16 experts (12 task-specific + 4 shared), each a 2-layer ReLU MLP:
        layer 1 feature-major on TensorE:  hT[H,B] = relu(W1.T @ xT + b1)
        layer 2 batch-major:               s[B,E]  = relu(hT.T @ W2)
    (hT is exactly the lhsT layer 2 needs, so expert outputs land in
    [B, E] orientation with no PE transposes at all)
  - layer-1 contraction split by precision: dims 0..511 in bf16, dims
    512..1023 via fp8(e4m3) DoubleRow matmuls at 0.5 cycles/row. The
    activations are split hi+lo (x ~= hi + lo, both e4m3, pre-scaled /4)
    so only the single-e4m3 W quantization error remains (~1.3e-2 rel
    end-to-end vs the 2e-2 gate); weights carry the matching x4 scale
  - gate logits computed in [B, n_exp] orientation; the fp8 dims use the
    exact hi+lo pair against bf16 gate weights (mixed-dtype matmul), so
    gates stay accurate; softmax along the free dim on ScalarE + DVE
  - each expert [B,E] tile is accumulated into per-task + shared-pool f32
    accumulators with one fused scalar_tensor_tensor (acc = s*g + acc) on
    DVE per contribution (the only engine whose ISA has the fused op)
  - emission order keeps PE continuously fed: shared experts first (their
    gate-combine is deferred via SBUF staging), input/weight DMAs ordered
    just-in-time, output flushed in chunks as the last contribution to
    each lands; the last expert runs progressively finer granules so only
    one b-tile's evict+combine+flush chain drains after the final matmul
"""

import numpy as np
import ml_dtypes

import concourse.bass as bass
import concourse.mybir as mybir
from concourse.tile import TileContext
from concourse.bass_utils import run_bass_kernel_spmd

BF16 = ml_dtypes.bfloat16
F8 = ml_dtypes.float8_e4m3fn

# Problem shapes (hardcoded per spec)
T, B, D, H, E = 3, 8192, 1024, 512, 256
S, NS = 4, 4
NCORES = 8
BL = B // NCORES          # per-core batch rows (1024)
NBT = BL // 128           # b-tiles of 128 per core (8)
KB = 4                    # bf16 contraction chunks (dims 0..511)
DSPL = 512                # first fp8 dim
NSL = 2                   # fp8 DoubleRow slices (256 dims each: 2 k-tiles)
KH = H // 128             # contraction chunks for layer 2 (4)
NEXP = T * S + NS         # 16 experts total
BN = 512                  # layer-1 moving free-dim chunk (1 PSUM bank of f32)

TRACE = False             # test harness sets kernel.TRACE = True for profiling
LAST_EXEC_NS = None

_CACHE = {}

# this walrus build rejects instructions carrying more than one semaphore wait
# condition ("Too many sync wait commands" in CoreV3 setupSyncWait; observed on
# Drain with 2+ and TensorTensor with 2), but Tile's sem-assigner and tail
# drain emit up to ~11 on one instruction
DRAIN_KEEP = 1
OTHER_KEEP = 1


def _split_excess_waits(nc):
    """Move overflow sem-waits onto same-engine NOPs inserted just before the
    overloaded instruction. Waiting earlier on the same engine preserves the
    ordering guarantee the wait provides."""
    n_split = 0
    for f in nc.m.functions:
        for bb in f.blocks:
            insts = bb.instructions
            need = False
            for i in insts:
                si = i.sync_info
                if si and si.on_wait and len(si.on_wait) > (
                    DRAIN_KEEP if isinstance(i, mybir.InstDrain) else OTHER_KEEP
                ):
                    need = True
                    break
            if not need:
                continue
            new_insts = []
            for inst in insts:
                si = inst.sync_info
                waits = list(si.on_wait) if si and si.on_wait else []
                keep = DRAIN_KEEP if isinstance(inst, mybir.InstDrain) else OTHER_KEEP
                if len(waits) > keep:
                    overflow = waits[: len(waits) - keep]
                    si.on_wait = waits[len(waits) - keep :]
                    for k, w in enumerate(overflow):
                        nop = mybir.InstNoOp(
                            name=f"{inst.name}-wsplit{k}", ins=[], outs=[]
                        )
                        nop.engine = inst.engine
                        nop.sync_info = mybir.SyncInfo(on_wait=[w], on_update=[])
                        new_insts.append(nop)
                        n_split += 1
                new_insts.append(inst)
            bb.instructions = new_insts
    return n_split


def _check_read_before_write(nc):
    """Emission-order lint: an on-chip tile read before any write means Tile
    will schedule the consumer against uninitialized memory."""
    import sys

    written = set()
    flagged = set()
    for f in nc.m.functions:
        for bb in f.blocks:
            for inst in bb.instructions:
                for arg in inst.ins:
                    t = getattr(getattr(arg, "bass_ap", None), "tensor", None)
                    name = getattr(t, "name", None)
                    if name and name not in written and name not in flagged:
                        space = getattr(t, "space", None)
                        if str(space) in ("MemorySpace.SBUF", "MemorySpace.PSUM"):
                            flagged.add(name)
                            print(
                                f"WARNING: {inst.name} reads {name} before any "
                                f"write (emission order)",
                                file=sys.stderr,
                            )
                for arg in inst.outs:
                    t = getattr(getattr(arg, "bass_ap", None), "tensor", None)
                    name = getattr(t, "name", None)
                    if name:
                        written.add(name)


def _build_program(with_b2=False, split_waits=True):
    f32 = mybir.dt.float32
    bf16 = mybir.dt.bfloat16
    fp8 = mybir.dt.float8e4
    relu = mybir.ActivationFunctionType.Relu
    expf = mybir.ActivationFunctionType.Exp
    mult = mybir.AluOpType.mult
    add = mybir.AluOpType.add
    DR = mybir.MatmulPerfMode.DoubleRow

    nc = bass.Bass()
    xT = nc.dram_tensor("xT", [4, DSPL, BL], bf16, kind="ExternalInput")
    x8h = nc.dram_tensor("x8h", [4, 128, NSL * 2 * BL], fp8, kind="ExternalInput")
    x8l = nc.dram_tensor("x8l", [4, 128, NSL * 2 * BL], fp8, kind="ExternalInput")
    w1 = nc.dram_tensor("w1", [NEXP, DSPL, H], bf16, kind="ExternalInput")
    w8 = nc.dram_tensor("w8", [NEXP, 128, NSL * 2 * H], fp8, kind="ExternalInput")
    w2 = nc.dram_tensor("w2", [NEXP, H, E], bf16, kind="ExternalInput")
    b1 = nc.dram_tensor("b1", [128, NEXP * KH], f32, kind="ExternalInput")
    wg = nc.dram_tensor("wg", [128, 4 * 8 * 16], bf16, kind="ExternalInput")
    bg = nc.dram_tensor("bg", [128, 4 * 16], f32, kind="ExternalInput")
    if with_b2:
        b2bc = nc.dram_tensor("b2bc", [128, NEXP * E], f32, kind="ExternalInput")
    out = nc.dram_tensor("out", [4, BL, E], f32, kind="ExternalOutput")

    with TileContext(nc) as tc:
        with (
            tc.tile_pool(name="const", bufs=1) as constp,
            tc.tile_pool(name="xp", bufs=1) as xp,
            tc.tile_pool(name="x8p", bufs=1) as x8p,
            tc.tile_pool(name="accp", bufs=1) as accp,
            tc.tile_pool(name="w1p", bufs=4) as w1p,
            tc.tile_pool(name="w8p", bufs=4) as w8p,
            tc.tile_pool(name="w2p", bufs=4) as w2p,
            tc.tile_pool(name="hp", bufs=3) as hp,
            tc.tile_pool(name="sp", bufs=4) as sp,
            tc.tile_pool(name="gp", bufs=4) as gp,
            tc.tile_pool(name="shp", bufs=8) as shp,
            tc.tile_pool(name="psh", bufs=3, space="PSUM") as psh_pool,
            tc.tile_pool(name="pss", bufs=5, space="PSUM") as pss_pool,
        ):
            wg_sb = constp.tile([128, 4 * 8 * 16], bf16)
            bg_sb = constp.tile([128, 4 * 16], f32)
            b1_sb = constp.tile([128, NEXP * KH], f32)
            if with_b2:
                b2_sb = constp.tile([128, NEXP * E], f32)

            xt_sb = [
                xp.tile([128, KB * BL], bf16, name=f"xt{src}") for src in range(4)
            ]
            x8h_sb = [
                x8p.tile([128, NSL * 2 * BL], fp8, name=f"x8h{src}")
                for src in range(4)
            ]
            x8l_sb = [
                x8p.tile([128, NSL * 2 * BL], fp8, name=f"x8l{src}")
                for src in range(4)
            ]

            def load_xt(src, half=None):
                # one DMA per 2-chunk half: HWDGE descriptor processing is a
                # shared ~625ns/DMA serial resource, so fewer+bigger wins
                for h in ([half] if half is not None else (0, 1)):
                    c0 = h * (KB // 2)
                    nc.sync.dma_start(
                        out=xt_sb[src][:, c0 * BL : (c0 + KB // 2) * BL].rearrange(
                            "p (c b) -> p c b", c=KB // 2
                        ),
                        in_=xT[src][c0 * 128 : (c0 + KB // 2) * 128, :].rearrange(
                            "(c p) b -> p c b", p=128
                        ),
                    )

            def load_x8(src):
                nc.sync.dma_start(out=x8h_sb[src], in_=x8h[src])
                nc.sync.dma_start(out=x8l_sb[src], in_=x8l[src])

            def x8_slice(x8sb, s, b0, b1_):
                # [128, kt=2, b] access pattern for DoubleRow rhs
                return x8sb[:, s * 2 * BL : (s + 1) * 2 * BL].rearrange(
                    "p (kt b) -> p kt b", kt=2
                )[:, :, b0:b1_]

            def w8_slice(w8sb, s, hc):
                # [128, kt=2, 128] access pattern for DoubleRow lhsT
                return w8sb[:, s * 2 * H : (s + 1) * 2 * H].rearrange(
                    "p (kt h) -> p kt h", kt=2
                )[:, :, hc * 128 : (hc + 1) * 128]

            acc = [accp.tile([128, NBT * E], f32, name=f"acc{t}") for t in range(4)]

            gate_sb = [
                constp.tile([128, NBT * 16], f32, name=f"gate{s}") for s in range(4)
            ]
            written = set()  # (acc_idx, bt) already initialized

            def emit_gates(src):
                wexp = 8 if src < 3 else 16
                # one psg region holds all NBT b-tiles of this gate set as a
                # single accumulation group. Blocks 0..3 are the bf16 x
                # chunks; blocks 4..7 are the fp8 (slice, ktile) dims, fed
                # with the exact hi+lo pair against bf16 gate weights (the
                # wg blocks carry the matching x4 scale).
                psg = pss_pool.tile([128, NBT * wexp], f32, name="psg", tag="ps_s")
                n_mm = (KB + NSL * 2 * 2) * NBT
                i_mm = 0
                for blk in range(8):
                    if blk < KB:
                        lhs_list = [
                            xt_sb[src][:, blk * BL + bt * 128 : blk * BL + bt * 128 + 128]
                            for bt in range(NBT)
                        ] * 1
                        lhs_iter = [(bt, lhs_list[bt]) for bt in range(NBT)]
                    else:
                        s, kt = divmod(blk - KB, 2)
                        lhs_iter = []
                        for x8sb in (x8h_sb[src], x8l_sb[src]):
                            base = s * 2 * BL + kt * BL
                            for bt in range(NBT):
                                lhs_iter.append(
                                    (bt, x8sb[:, base + bt * 128 : base + bt * 128 + 128])
                                )
                    for bt, lhs in lhs_iter:
                        nc.tensor.matmul(
                            psg[:, bt * wexp : bt * wexp + wexp],
                            lhsT=lhs,
                            rhs=wg_sb[:, (src * 8 + blk) * 16 : (src * 8 + blk) * 16 + wexp],
                            start=(i_mm == 0),
                            stop=(i_mm == n_mm - 1),
                        )
                        i_mm += 1
                logits = gp.tile([128, NBT * wexp], f32, tag="logits")
                nc.scalar.copy(logits, psg)
                for bt in range(NBT):
                    logit = gp.tile([128, 16], f32, tag="logit")
                    nc.vector.tensor_add(
                        logit[:, :wexp],
                        logits[:, bt * wexp : bt * wexp + wexp],
                        bg_sb[:, src * 16 : src * 16 + wexp],
                    )
                    g_ap = gate_sb[src][:, bt * 16 : bt * 16 + wexp]
                    ssum = gp.tile([128, 1], f32, tag="ssum")
                    nc.scalar.activation(g_ap, logit[:, :wexp], expf, accum_out=ssum)
                    rsum = gp.tile([128, 1], f32, tag="rsum")
                    nc.vector.reciprocal(rsum, ssum)
                    nc.vector.tensor_scalar_mul(g_ap, g_ap, rsum)

            # SBUF staging for shared experts computed before the task gates
            # exist: their [B,E] tiles wait here until the gates are ready
            stage = {}

            def load_w1(e):
                w1_sb = w1p.tile([128, KB * H], bf16, name="w1_sb", tag="w1_sb")
                nc.sync.dma_start(
                    out=w1_sb.rearrange("p (c h) -> p c h", c=KB),
                    in_=w1[e].rearrange("(c p) h -> p c h", p=128),
                )
                return w1_sb

            def load_w8(e):
                w8_sb = w8p.tile([128, NSL * 2 * H], fp8, name="w8_sb", tag="w8_sb")
                nc.sync.dma_start(out=w8_sb, in_=w8[e])
                return w8_sb

            def load_w2(e):
                w2_sb = w2p.tile([128, KH * E], bf16, name="w2_sb", tag="w2_sb")
                nc.sync.dma_start(
                    out=w2_sb.rearrange("p (c f) -> p c f", c=KH),
                    in_=w2[e].rearrange("(c p) f -> p c f", p=128),
                )
                return w2_sb

            def contribs_of(e):
                if e < T * S:
                    t, s = divmod(e, S)
                    return [(3, t * S + s), (t, s)]
                jsh = e - T * S
                return [(t, S + jsh) for t in range(T)] + [(3, T * S + jsh)]

            def emit_contrib(gset, col, bt, src_tile):
                # combines live on DVE: the only engine whose ISA has the
                # fused scalar_tensor_tensor (acc = s*g + acc); Pool cannot
                # even read PSUM on real TRN2
                g = gate_sb[gset][:, bt * 16 + col : bt * 16 + col + 1]
                a = acc[gset][:, bt * E : (bt + 1) * E]
                if (gset, bt) not in written:
                    written.add((gset, bt))
                    nc.vector.tensor_scalar_mul(a, src_tile, g)
                else:
                    nc.vector.scalar_tensor_tensor(
                        a, src_tile, g, a, op0=mult, op1=add
                    )

            def emit_contribs(e, bt, src_tile):
                for gset, col in contribs_of(e):
                    emit_contrib(gset, col, bt, src_tile)

            def flush(t, row0, nrows):
                nb = nrows // 128
                b0 = row0 // 128
                nc.sync.dma_start(
                    out=out[t][row0 : row0 + nrows, :].rearrange(
                        "(b p) f -> p b f", p=128
                    ),
                    in_=acc[t][:, b0 * E : (b0 + nb) * E].rearrange(
                        "p (b f) -> p b f", b=nb
                    ),
                )

            def emit_l1_group(ps_h, w1_sb, w8_sb, src, hc, off, W):
                """One layer-1 accumulation group: 4 bf16 chunks + 2 fp8
                DoubleRow slices x (hi, lo)."""
                for c in range(KB):
                    nc.tensor.matmul(
                        ps_h,
                        lhsT=w1_sb[:, c * H + hc * 128 : c * H + hc * 128 + 128],
                        rhs=xt_sb[src][:, c * BL + off : c * BL + off + W],
                        start=(c == 0),
                        stop=False,
                    )
                for s in range(NSL):
                    for x8sb in (x8h_sb[src], x8l_sb[src]):
                        nc.tensor.matmul(
                            ps_h,
                            lhsT=w8_slice(w8_sb, s, hc),
                            rhs=x8_slice(x8sb, s, off, off + W),
                            start=False,
                            stop=(s == NSL - 1 and x8sb is x8l_sb[src]),
                            perf_mode=DR,
                        )

            DEFAULT_BN = [(0, BN), (BN, BN)]
            # last expert: progressively finer granules so only ONE b-tile's
            # evict+combine+flush chain drains after the last matmul
            TAIL_BN = [(0, BN), (BN, BN // 2), (768, 128), (896, 128)]

            def emit_expert(e, src, finalize, defer=False, extra_per_bt=None,
                            w1_pre=None, w8_pre=None, h_pre=None, w2_pre=None,
                            fine_tail=False):
                w1_sb = w1_pre if w1_pre is not None else load_w1(e)
                w8_sb = w8_pre if w8_pre is not None else load_w8(e)
                w2_sb = w2_pre if w2_pre is not None else load_w2(e)

                for off, W in (TAIL_BN if fine_tail else DEFAULT_BN):
                    if h_pre is not None and off == 0:
                        h_sb = h_pre
                    else:
                        h_sb = hp.tile([128, KH * W], bf16, name="h_sb", tag="h_sb")
                        for hc in range(KH):
                            ps_h = psh_pool.tile([128, W], f32, name="ps_h", tag="ps_h")
                            emit_l1_group(ps_h, w1_sb, w8_sb, src, hc, off, W)
                            nc.scalar.activation(
                                h_sb[:, hc * W : (hc + 1) * W],
                                ps_h,
                                relu,
                                bias=b1_sb[:, e * KH + hc : e * KH + hc + 1],
                            )
                    for j in range(W // 128):
                        bt = off // 128 + j
                        ps_s = pss_pool.tile([128, E], f32, name="ps_s", tag="ps_s")
                        for hc in range(KH):
                            nc.tensor.matmul(
                                ps_s,
                                lhsT=h_sb[:, hc * W + j * 128 : hc * W + j * 128 + 128],
                                rhs=w2_sb[:, hc * E : (hc + 1) * E],
                                start=(hc == 0),
                                stop=(hc == KH - 1),
                            )
                        if with_b2:
                            nc.vector.tensor_add(
                                ps_s, ps_s, b2_sb[:, e * E : (e + 1) * E]
                            )
                        if defer:
                            st = shp.tile([128, E], bf16, name=f"st{e}", tag=f"st{e}")
                            nc.scalar.activation(st, ps_s, relu)
                            stage[(e, bt)] = st
                            if extra_per_bt is not None:
                                extra_per_bt(bt)
                        else:
                            s_sb = sp.tile([128, E], bf16, name="s_sb", tag="s_sb")
                            if fine_tail and off == TAIL_BN[-1][0]:
                                # final b-tile: relu-evict on DVE (max(x,0)
                                # straight from PSUM) so the tail chain is
                                # DVE-serial with no Act sem hop
                                nc.vector.tensor_scalar_max(s_sb, ps_s, 0.0)
                            else:
                                nc.scalar.activation(s_sb, ps_s, relu)
                            # deferred shared-expert combines first: they only
                            # need staged tiles + gates, so they never sit on
                            # the critical tail chain
                            if extra_per_bt is not None:
                                extra_per_bt(bt)
                            emit_contribs(e, bt, s_sb)

                        # flush finished accumulator rows to DRAM as soon as
                        # their last contribution lands (per bn/granule chunk)
                        if j == W // 128 - 1:
                            for t in finalize:
                                flush(t, off, W)

            # Emission = per-engine program order (modulo the Tile list
            # scheduler). Shared experts first: they need no gates at compute
            # time (combine deferred via SBUF staging), then gates(3), the
            # remaining shared experts, then task phases 0/1/2 with the
            # deferred shared-pool contributions interleaved per b-tile.
            finalize_at = {3: [0], 7: [1], 11: [2, 3]}

            # DMA prologue: expert 12's first layer-1 eats (w1 + x-half) near
            # the DMA link rate, so pieces grow (1 chunk, 3 chunks) for an
            # early PE start, with the fp8 pieces right behind (their matmuls
            # run after the bf16 c-loop), then second halves + w2[12].
            w1_12 = w1p.tile([128, KB * H], bf16, name="w1_sb", tag="w1_sb")
            nc.sync.dma_start(
                out=w1_12[:, 0:H], in_=w1[12][0:128, :],
            )
            nc.sync.dma_start(
                out=xt_sb[3][:, 0:BN], in_=xT[3][0:128, 0:BN],
            )
            nc.sync.dma_start(out=b1_sb, in_=b1[:, :])
            nc.sync.dma_start(
                out=w1_12[:, H : KB * H].rearrange("p (c h) -> p c h", c=KB - 1),
                in_=w1[12][128 : KB * 128, :].rearrange("(c p) h -> p c h", p=128),
            )
            nc.sync.dma_start(
                out=xt_sb[3][:, BL : KB * BL].rearrange(
                    "p (c b) -> p c b", c=KB - 1
                )[:, :, 0:BN],
                in_=xT[3][128 : KB * 128, 0:BN].rearrange("(c p) b -> p c b", p=128),
            )
            w8_12 = load_w8(12)
            # fp8 x pieces for the first-half columns (hi then lo)
            nc.sync.dma_start(
                out=x8h_sb[3].rearrange("p (sk b) -> p sk b", sk=NSL * 2)[:, :, 0:BN],
                in_=x8h[3].rearrange("p (sk b) -> p sk b", sk=NSL * 2)[:, :, 0:BN],
            )
            nc.sync.dma_start(
                out=x8l_sb[3].rearrange("p (sk b) -> p sk b", sk=NSL * 2)[:, :, 0:BN],
                in_=x8l[3].rearrange("p (sk b) -> p sk b", sk=NSL * 2)[:, :, 0:BN],
            )
            nc.sync.dma_start(out=wg_sb, in_=wg[:, :])
            w2_12 = load_w2(12)
            # second halves
            nc.sync.dma_start(
                out=xt_sb[3].rearrange("p (c b) -> p c b", c=KB)[:, :, BN:BL],
                in_=xT[3].rearrange("(c p) b -> p c b", p=128)[:, :, BN:BL],
            )
            nc.sync.dma_start(
                out=x8h_sb[3].rearrange("p (sk b) -> p sk b", sk=NSL * 2)[:, :, BN:BL],
                in_=x8h[3].rearrange("p (sk b) -> p sk b", sk=NSL * 2)[:, :, BN:BL],
            )
            nc.sync.dma_start(
                out=x8l_sb[3].rearrange("p (sk b) -> p sk b", sk=NSL * 2)[:, :, BN:BL],
                in_=x8l[3].rearrange("p (sk b) -> p sk b", sk=NSL * 2)[:, :, BN:BL],
            )
            nc.sync.dma_start(out=bg_sb, in_=bg[:, :])
            if with_b2:
                nc.sync.dma_start(out=b2_sb, in_=b2bc[:, :])

            # PE prologue: expert 12's first-half layer-1 runs c-outer across
            # 3 PSUM banks so PE consumes each (w1, x) chunk pair as it
            # lands; the fp8 DoubleRow passes append to each group; the 4th
            # h-chunk runs from a spare pss bank (no eviction wait).
            h12 = hp.tile([128, KH * BN], bf16, name="h12", bufs=1)
            ph = [
                psh_pool.tile([128, BN], f32, name=f"ph{hc}", tag="ps_h")
                for hc in range(3)
            ]
            for c in range(KB):
                for hc in range(3):
                    nc.tensor.matmul(
                        ph[hc],
                        lhsT=w1_12[:, c * H + hc * 128 : c * H + hc * 128 + 128],
                        rhs=xt_sb[3][:, c * BL : c * BL + BN],
                        start=(c == 0),
                        stop=False,
                    )
            for s in range(NSL):
                for x8sb in (x8h_sb[3], x8l_sb[3]):
                    last = s == NSL - 1 and x8sb is x8l_sb[3]
                    for hc in range(3):
                        nc.tensor.matmul(
                            ph[hc],
                            lhsT=w8_slice(w8_12, s, hc),
                            rhs=x8_slice(x8sb, s, 0, BN),
                            start=False,
                            stop=last,
                            perf_mode=DR,
                        )
            for hc in range(3):
                nc.scalar.activation(
                    h12[:, hc * BN : (hc + 1) * BN], ph[hc], relu,
                    bias=b1_sb[:, 12 * KH + hc : 12 * KH + hc + 1],
                )
            ph3 = pss_pool.tile([128, BN], f32, name="ph3", tag="ps_s")
            emit_l1_group(ph3, w1_12, w8_12, 3, 3, 0, BN)
            nc.scalar.activation(
                h12[:, 3 * BN : 4 * BN], ph3, relu,
                bias=b1_sb[:, 12 * KH + 3 : 12 * KH + 4],
            )

            emit_expert(12, 3, [], defer=True, w1_pre=w1_12, w8_pre=w8_12,
                        h_pre=h12, w2_pre=w2_12)
            emit_gates(3)
            emit_expert(13, 3, [], defer=True)
            emit_expert(14, 3, [], defer=True)
            load_xt(0)
            load_x8(0)
            emit_expert(15, 3, [], defer=True)
            emit_gates(0)

            # deferred shared-expert combines, spread across the task phases:
            # st_{12+k} -> acc_t runs during phase t's k-th expert (so
            # gates(1)/gates(2) aren't needed until their own phase), and
            # st_{12+k} -> acc3 runs during phase 0.
            def make_hook(shared_e, phase):
                def hook(bt):
                    emit_contrib(phase, S + (shared_e - T * S), bt,
                                 stage[(shared_e, bt)])
                    if phase == 0:
                        emit_contrib(3, T * S + (shared_e - T * S), bt,
                                     stage[(shared_e, bt)])
                return hook

            for k, e in enumerate([0, 1, 2, 3]):
                emit_expert(e, 0, finalize_at.get(e, []),
                            extra_per_bt=make_hook(12 + k, 0))
                if e == 1:
                    load_xt(1, half=0)
                if e == 2:
                    load_xt(1, half=1)
                    load_x8(1)
                    emit_gates(1)
            for k, e in enumerate([4, 5, 6, 7]):
                emit_expert(e, 1, finalize_at.get(e, []),
                            extra_per_bt=make_hook(12 + k, 1))
                if e == 5:
                    load_xt(2, half=0)
                if e == 6:
                    load_xt(2, half=1)
                    load_x8(2)
                    emit_gates(2)
            for k, e in enumerate([8, 9, 10, 11]):
                emit_expert(e, 2, finalize_at.get(e, []),
                            extra_per_bt=make_hook(12 + k, 2),
                            fine_tail=(e == 11))

    _check_read_before_write(nc)
    if split_waits:
        _split_excess_waits(nc)
    return nc


def _prep_shared(W_spec1, b_spec1, W_spec2, b_spec2, W_sh1, b_sh1, W_sh2, b_sh2,
                 W_gate, b_gate, W_gate_sh, b_gate_sh, with_b2):
    """Host-side prep of the replicated (per-core-identical) tensors."""
    W1_all = np.concatenate([W_spec1, W_sh1], axis=0).astype(np.float32)
    w1 = np.ascontiguousarray(W1_all[:, :DSPL, :].astype(BF16))
    # fp8 upper-half weights, x4 pre-scale (pairs with the x/4 activations),
    # layout [e][p][(slice, ktile, h)] for the DoubleRow lhsT
    w8 = np.ascontiguousarray(
        (W1_all[:, DSPL:, :] * 4.0)
        .astype(F8)
        .reshape(NEXP, NSL, 2, 128, H)
        .transpose(0, 3, 1, 2, 4)
        .reshape(NEXP, 128, NSL * 2 * H)
    )
    w2 = np.ascontiguousarray(
        np.concatenate([W_spec2, W_sh2], axis=0).astype(BF16)
    )
    # layer-1 biases, batched: [128, e*KH + hc] (partition = h within chunk)
    b1 = np.ascontiguousarray(
        np.concatenate([b_spec1, b_sh1], axis=0)
        .astype(np.float32)
        .reshape(NEXP, KH, 128)
        .transpose(2, 0, 1)
        .reshape(128, NEXP * KH)
    )
    # gate weights: [4 sets, D, 16] (task gates padded 8 -> 16 with zeros);
    # blocks 4..7 (dims 512..1023) are consumed against the /4-scaled fp8
    # activations, so they carry the compensating x4 scale
    wg_full = np.zeros((4, D, 16), np.float32)
    wg_full[:T, :, : S + NS] = W_gate
    wg_full[3] = W_gate_sh
    wg_blocks = wg_full.reshape(4, 8, 128, 16).copy()
    wg_blocks[:, KB:] *= 4.0
    wg = np.ascontiguousarray(
        wg_blocks.transpose(2, 0, 1, 3).reshape(128, 4 * 8 * 16)
    ).astype(BF16)
    bg_full = np.zeros((4, 16), np.float32)
    bg_full[:T, : S + NS] = b_gate
    bg_full[3] = b_gate_sh
    bg = np.ascontiguousarray(
        np.broadcast_to(bg_full[None], (128, 4, 16)).reshape(128, 64)
    )
    res = dict(w1=w1, w8=w8, w2=w2, b1=b1, wg=wg, bg=bg)
    if with_b2:
        b2_full = np.concatenate([b_spec2, b_sh2], axis=0).astype(np.float32)
        res["b2bc"] = np.ascontiguousarray(
            np.broadcast_to(b2_full.reshape(1, NEXP * E), (128, NEXP * E))
        )
    return res


def kernel(x_tasks, x_shared, W_spec1, b_spec1, W_spec2, b_spec2,
           W_sh1, b_sh1, W_sh2, b_sh2, W_gate, b_gate, W_gate_sh, b_gate_sh):
    global LAST_EXEC_NS
    with_b2 = bool(np.any(np.asarray(b_spec2)) or np.any(np.asarray(b_sh2)))
    key = ("nc", with_b2)
    if key not in _CACHE:
        _CACHE[key] = _build_program(with_b2=with_b2)
    nc = _CACHE[key]
    _CACHE["nc"] = nc  # latest program, for the test harness's TimelineSim

    shared = _prep_shared(W_spec1, b_spec1, W_spec2, b_spec2, W_sh1, b_sh1,
                          W_sh2, b_sh2, W_gate, b_gate, W_gate_sh, b_gate_sh,
                          with_b2)

    x_tasks = np.asarray(x_tasks, np.float32)
    x_shared = np.asarray(x_shared, np.float32)

    in_maps = []
    for i in range(NCORES):
        sl = slice(i * BL, (i + 1) * BL)
        xt = np.empty((4, DSPL, BL), BF16)
        xh = np.empty((4, 128, NSL * 2 * BL), F8)
        xl = np.empty((4, 128, NSL * 2 * BL), F8)
        srcs = [x_tasks[0, sl], x_tasks[1, sl], x_tasks[2, sl], x_shared[sl]]
        for s_i, xsrc in enumerate(srcs):
            xt[s_i] = xsrc[:, :DSPL].T.astype(BF16)
            xs = (xsrc[:, DSPL:] / 4.0).astype(np.float32)   # [BL, 512]
            hi = xs.astype(F8)
            lo = (xs - hi.astype(np.float32)).astype(F8)
            # layout [p][(slice, ktile, b)]
            xh[s_i] = (
                hi.reshape(BL, NSL, 2, 128).transpose(3, 1, 2, 0)
                .reshape(128, NSL * 2 * BL)
            )
            xl[s_i] = (
                lo.reshape(BL, NSL, 2, 128).transpose(3, 1, 2, 0)
                .reshape(128, NSL * 2 * BL)
            )
        m = {"xT": xt, "x8h": xh, "x8l": xl}
        m.update(shared)
        in_maps.append(m)

    res = run_bass_kernel_spmd(nc, in_maps, core_ids=list(range(NCORES)), trace=TRACE)
    LAST_EXEC_NS = res.exec_time_ns

    full = np.empty((T + 1, B, E), np.float32)
    for i in range(NCORES):
        full[:, i * BL : (i + 1) * BL, :] = res.results[i]["out"]
    return full
